# revision 51
# baseline (speedup 1.0000x reference)
"""Trainium2 Bass kernel for nn_Align_54279796687162 (sparse_attention).

Pure data parallel: one sample per NeuronCore (B=8 over 8 cores).
Per-core layout: activations channel-major [C(partitions), n = h*128 + w].
bf16 matmul inputs, f32 PSUM accumulation.

v3 structure:
 - bf16 input/output DMA (host casts); weights prefetched after the input.
 - Phase A: conv3x3 in 12-row slabs; cb^T / cf^T via DRAM-sourced xbar
   transposes on the scalar HWDGE queue (decoupled from bulk traffic);
   energy matmuls trail the transposes by 3 slabs; shunts on DVE;
   softmax -> rank-16 correction matrices A so xf is never materialized.
 - qkv: 512-col PSUM chunks (4 tensors x 1 bank, bufs=2); the 4 rank-16
   corrections issue back-to-back at distinct 32-row strips (concurrent
   PE subarrays); q/k extracted by scalar into a PADDED DRAM layout
   (contiguous depthwise window reloads), v0/v1 by DVE into padded SBUF.
 - Depthwise 3x3 in 12-row superblocks (diag weight loads amortized over
   4 groups; groups q,k,v0 on tensor, v1 as lazily-emitted DVE FMA
   chains), pointwise, then per-4-row-block gating straight from SBUF
   (no qo DRAM roundtrip), axial attention between qkv and depthwise.

Self-contained: hardcodes shapes, builds the Bass/Tile graph, shards inputs,
runs via run_bass_kernel_spmd on cores 0-7, gathers the full output.
"""

import numpy as np
import ml_dtypes

import concourse.bass as bass
import concourse.mybir as mybir
import concourse.tile as tile
from concourse import bacc
from concourse.bass_utils import run_bass_kernel_spmd

BF = mybir.dt.bfloat16
F32 = mybir.dt.float32
AF = mybir.ActivationFunctionType
ALU = mybir.AluOpType
AX = mybir.AxisListType

H = W = 128
N = H * W            # 16384
BL = 512             # block size (4 rows * 128)
CH = 4               # chunks
SCALE = 0.25         # KD ** -0.5
PST = 132            # padded row stride for q/k/v (DW conv layout)
PSZ = PST * 130      # padded tensor size per partition
SLAB = 12            # conv slab rows

# depthwise groups 0..DW_TENSOR_GROUPS-1 (of q,k,v0,v1) run as diag matmuls
# on the tensor engine; the rest run as DVE FMA chains.
DW_TENSOR_GROUPS = 3

# bias column map in the packed [128, 20] f32 bias tile
B_CCAM, B_ENC, B_Q, B_K, B_V, B_DW, B_PW, B_ROW, B_COL, B_PROJ3 = (
    0, 2, 3, 4, 5, 7, 11, 13, 15, 17)

_CACHE = {}


def _ap(base, extra_off, free_dims):
    """Build an AP from a tile's base AP with custom free dims."""
    b = base[:]
    return bass.AP(b.tensor, b.offset + extra_off, [list(b.ap[0])] + free_dims)


def build_graph(scale_ccam: float):
    nc = bacc.Bacc(None, target_bir_lowering=False)

    xb = nc.dram_tensor("xb", [128, N], BF, kind="ExternalInput")
    w3t = nc.dram_tensor("w3t", [128, 9 * 256], BF, kind="ExternalInput")
    wenc = nc.dram_tensor("wenc", [128, 32], BF, kind="ExternalInput")
    wq = nc.dram_tensor("wq", [128, 256], BF, kind="ExternalInput")
    wk = nc.dram_tensor("wk", [128, 256], BF, kind="ExternalInput")
    wv = nc.dram_tensor("wv", [128, 512], BF, kind="ExternalInput")
    wqs = nc.dram_tensor("wqs", [128, 256], BF, kind="ExternalInput")
    wks = nc.dram_tensor("wks", [128, 256], BF, kind="ExternalInput")
    wvs = nc.dram_tensor("wvs", [128, 512], BF, kind="ExternalInput")
    dwd = nc.dram_tensor("dwd", [128, 36 * 128], BF, kind="ExternalInput")
    wpw = nc.dram_tensor("wpw", [128, 4 * 256], BF, kind="ExternalInput")
    wrow = nc.dram_tensor("wrow", [128, 512], BF, kind="ExternalInput")
    wcol = nc.dram_tensor("wcol", [128, 512], BF, kind="ExternalInput")
    wproj = nc.dram_tensor("wproj", [128, 512], BF, kind="ExternalInput")
    post = nc.dram_tensor("post", [16, 4 * 512], BF, kind="ExternalInput")
    interpm = nc.dram_tensor("interpm", [16, 128], BF, kind="ExternalInput")
    identb = nc.dram_tensor("identb", [128, 128], BF, kind="ExternalInput")
    identf = nc.dram_tensor("identf", [128, 128], F32, kind="ExternalInput")
    onesb = nc.dram_tensor("onesb", [128, 1], BF, kind="ExternalInput")
    biases = nc.dram_tensor("biases", [128, 20], F32, kind="ExternalInput")
    dwsc = nc.dram_tensor("dwsc", [128, 36], F32, kind="ExternalInput")

    cb_dram = nc.dram_tensor("cb_dram", [2, 128, N], BF, kind="Internal")
    cf_dram = nc.dram_tensor("cf_dram", [16, N], BF, kind="Internal")
    qk_dram = nc.dram_tensor("qk_dram", [2, 128, PSZ], BF, kind="Internal")
    out = nc.dram_tensor("out", [256, N], BF, kind="ExternalOutput")

    # conv slab row-starts: 10 slabs of 12 rows + 1 slab of 8
    slabs = [(s * SLAB, SLAB) for s in range(10)] + [(120, 8)]
    taps = [(1, 1), (0, 1), (2, 1), (1, 0), (1, 2),
            (0, 0), (0, 2), (2, 0), (2, 2)]

    with tile.TileContext(nc) as tc:
      with tc.tile_pool(name="cst", bufs=1) as cst:
        wenc_s = cst.tile([128, 32], BF)
        wq_s = cst.tile([128, 256], BF)
        wk_s = cst.tile([128, 256], BF)
        wv_s = cst.tile([128, 512], BF)
        wqs_s = cst.tile([128, 256], BF)
        wks_s = cst.tile([128, 256], BF)
        wvs_s = cst.tile([128, 512], BF)
        wpw_s = cst.tile([128, 4 * 256], BF)
        wrow_s = cst.tile([128, 512], BF)
        wcol_s = cst.tile([128, 512], BF)
        wproj_s = cst.tile([128, 512], BF)
        post_s = cst.tile([16, 4 * 512], BF)
        interp_s = cst.tile([16, 128], BF)
        idb_s = cst.tile([128, 128], BF)
        ones_s = cst.tile([128, 1], BF)
        bia_s = cst.tile([128, 20], F32)
        dwsc_s = cst.tile([128, 36], F32)
        for t, d in [(bia_s, biases), (wenc_s, wenc), (idb_s, identb)]:
            nc.sync.dma_start(t[:], d[:])

        def load_late_consts():
            # issued after the xpad chunks so they don't delay slab 0
            for t, d in [(wq_s, wq), (wk_s, wk),
                         (wv_s, wv), (wqs_s, wqs), (wks_s, wks),
                         (wvs_s, wvs), (wpw_s, wpw), (wrow_s, wrow),
                         (wcol_s, wcol), (wproj_s, wproj), (post_s, post),
                         (interp_s, interpm),
                         (ones_s, onesb), (dwsc_s, dwsc)]:
                nc.sync.dma_start(t[:], d[:])

        # persistent small tensors produced in phase A, consumed later
        xfs_row = [cst.tile([128, 512], BF, tag=f"xfsr{h}", name=f"xfsr{h}")
                   for h in range(2)]
        xfs_col = [cst.tile([128, 512], F32, tag=f"xfsc{h}", name=f"xfsc{h}")
                   for h in range(2)]
        cfs_row = cst.tile([16, 512], F32)
        cfs_col = cst.tile([16, 512], F32)
        cfs_row_b = cst.tile([16, 512], BF)
        cfs_col_b = cst.tile([16, 512], BF)
        at_h = [cst.tile([128, 16], BF, tag=f"at{h}", name=f"at{h}")
                for h in range(2)]
        # correction matrices A^T [16, 128]: q, k, v0, v1 (normal + shunt).
        # normal set packed at partition offsets 32*i for tile_position use.
        A4n = cst.tile([128, 128], BF)
        A_s = [cst.tile([16, 128], BF, tag=f"As{i}", name=f"As{i}")
               for i in range(4)]
        xproj = {(d_, t_): cst.tile([128, 512], BF, tag=f"xp{d_}{t_}",
                                    name=f"xp{d_}{t_}")
                 for d_ in range(2) for t_ in range(2)}

        # =========================================================
        # Phase A: conv3x3 slabs; cb^T/cf^T via DMA transpose;
        # energy; shunts; softmax; A matrices
        # =========================================================
        with (
            tc.tile_pool(name="pa", bufs=1) as pa,
            tc.tile_pool(name="pasl", bufs=3) as pasl,
            tc.tile_pool(name="par", bufs=3) as par,
            tc.tile_pool(name="pamm", bufs=2, space="PSUM") as pamm,
            tc.tile_pool(name="pacf", bufs=2, space="PSUM") as pacf,
            tc.tile_pool(name="pae", bufs=1, space="PSUM") as pae,
            tc.tile_pool(name="pasm", bufs=1, space="PSUM") as pasm,
        ):
            xpad = pa.tile([128, 130 * 130], BF)
            w3_s = pa.tile([128, 9 * 256], BF)
            idf_s = pa.tile([128, 128], F32)
            nc.sync.dma_start(w3_s[:], w3t[:])
            nc.sync.dma_start(idf_s[:], identf[:])

            # pad borders only; interior filled by strided HWDGE chunk loads
            nc.vector.memset(_ap(xpad, 0, [[1, 130]]), 0.0)
            nc.vector.memset(_ap(xpad, 129 * 130, [[1, 130]]), 0.0)
            nc.vector.memset(_ap(xpad, 129, [[130, 129], [1, 2]]), 0.0)
            for rc in range(8):
                nc.sync.dma_start(
                    _ap(xpad, 131 + rc * 16 * 130, [[130, 16], [1, 128]]),
                    xb[:, rc * 2048:(rc + 1) * 2048])
            load_late_consts()

            e_ps = pae.tile([16, 256], F32)

            first_mm = [True]
            pend = []   # (cbT, cfT, srows) pending energy MMs, 1-slab delay

            def emit_energy(last):
                cbT, cfT, srows = pend.pop(0)
                for j in range(srows):
                    for half in range(2):
                        nc.tensor.matmul(
                            e_ps[:, half * 128:(half + 1) * 128],
                            cfT[:, j, :], cbT[half][:, j, :],
                            start=first_mm[0],
                            stop=(last and j == srows - 1 and half == 1))
                        first_mm[0] = False

            for si, (s0, srows) in enumerate(slabs):
                # --- conv3x3 for this slab, both halves ---
                grp = [(r0, min(3, s0 + srows - r0))
                       for r0 in range(s0, s0 + srows, 3)]
                cbs = [pasl.tile([128, SLAB * 128], BF, tag=f"cbs{h}",
                                 name=f"cbs{h}") for h in range(2)]
                for half in range(2):
                    for g0 in range(0, len(grp), 2):
                        pg = grp[g0:g0 + 2]
                        ps = pamm.tile([128, 1024], F32, tag="amm")
                        for t9 in range(9):
                            ky, kx = divmod(t9, 3)
                            for j, (r0, nr) in enumerate(pg):
                                rhs = _ap(xpad, (r0 + ky) * 130 + kx,
                                          [[1, nr * 130 - 2]])
                                nc.tensor.matmul(
                                    _ap(ps, j * 512, [[1, nr * 130 - 2]]),
                                    w3_s[:, t9 * 256 + half * 128:
                                         t9 * 256 + half * 128 + 128],
                                    rhs, start=(t9 == 0), stop=(t9 == 8))
                        nrows = sum(nr for _, nr in pg)
                        if len(pg) == 2:
                            src = _ap(ps, 0, [[512, 2], [130, pg[0][1]],
                                              [1, 128]])
                        else:
                            src = _ap(ps, 0, [[130, pg[0][1]], [1, 128]])
                        lr = pg[0][0] - s0
                        nc.scalar.activation(
                            cbs[half][:, lr * 128:(lr + nrows) * 128],
                            src, AF.Relu,
                            bias=bia_s[:, B_CCAM + half:B_CCAM + half + 1])
                    nc.sync.dma_start(
                        cb_dram[half, :, s0 * 128:(s0 + srows) * 128],
                        cbs[half][:, 0:srows * 128])

                # --- cf for this slab: relu(wenc @ cb + b_enc) ---
                nbl = (srows * 128) // 512
                cfsl = pasl.tile([16, SLAB * 128], BF, tag="cfsl")
                for b in range(nbl):
                    ps = pacf.tile([16, 512], F32, tag="acf")
                    for half in range(2):
                        nc.tensor.matmul(
                            ps[:], wenc_s[:, half * 16:half * 16 + 16],
                            cbs[half][:, b * 512:(b + 1) * 512],
                            start=(half == 0), stop=(half == 1))
                    nc.scalar.activation(
                        cfsl[:, b * 512:(b + 1) * 512], ps[:], AF.Relu,
                        bias=bia_s[:16, B_ENC:B_ENC + 1])
                nc.sync.dma_start(
                    cf_dram[:, s0 * 128:(s0 + srows) * 128],
                    cfsl[:, 0:srows * 128])

                # --- transposes via DMA xbar ---
                cbT = [pasl.tile([128, SLAB, 128], BF, tag=f"cbT{h}",
                                 name=f"cbT{h}", bufs=4) for h in range(2)]
                cfT = pasl.tile([128, SLAB, 16], BF, tag="cfT", bufs=4)
                # alternate HWDGE rings by slab so transpose descriptors
                # drain from two queues concurrently
                teng = nc.scalar if si % 2 == 0 else nc.sync
                for half in range(2):
                    teng.dma_start(
                        cbT[half][:, 0:srows, :],
                        cb_dram[half, :, s0 * 128:(s0 + srows) * 128],
                        transpose=True)
                teng.dma_start(cfT[:, 0:srows, :],
                               cf_dram[:, s0 * 128:(s0 + srows) * 128],
                               transpose=True)

                # --- energy accumulation, pipelined one slab behind ---
                pend.append((cbT, cfT, srows))
                if si >= 3:
                    emit_energy(last=False)

                # --- shunts of cb (into xfs_*) and cf (into cfs_*) ---
                for b4 in range(srows // 4):
                    b = (s0 // 4) + b4
                    lr = b4 * 4
                    ci = b // 8
                    for half in range(2):
                        with nc.allow_low_precision(reason="bf16 shunt sums"):
                            src = _ap(cbs[half], lr * 128,
                                      [[1, 4], [128, 4], [4, 32]])
                            dst = _ap(xfs_row[half], 4 * b,
                                      [[128, 4], [1, 4]])
                            nc.vector.tensor_reduce(dst, src, axis=AX.X,
                                                    op=ALU.add)
                        part = par.tile([128, 128], F32, tag=f"cp{half}",
                                        name=f"cp{half}", bufs=2)
                        src = _ap(cbs[half], lr * 128, [[1, 128], [128, 4]])
                        nc.vector.tensor_reduce(part[:], src, axis=AX.X,
                                                op=ALU.add)
                        dstc = xfs_col[half][:, ci * 128:(ci + 1) * 128]
                        if b % 8 == 0:
                            nc.gpsimd.tensor_copy(dstc, part[:])
                        else:
                            nc.gpsimd.tensor_tensor(dstc, dstc, part[:],
                                                    ALU.add)
                    # cf shunts
                    src = _ap(cfsl, lr * 128, [[1, 4], [128, 4], [4, 32]])
                    dst = _ap(cfs_row, 4 * b, [[128, 4], [1, 4]])
                    nc.vector.tensor_reduce(dst, src, axis=AX.X, op=ALU.add)
                    partf = par.tile([16, 128], F32, tag="cpf", bufs=2)
                    src = _ap(cfsl, lr * 128, [[1, 128], [128, 4]])
                    nc.vector.tensor_reduce(partf[:], src, axis=AX.X,
                                            op=ALU.add)
                    dstc = cfs_col[:, ci * 128:(ci + 1) * 128]
                    if b % 8 == 0:
                        nc.gpsimd.tensor_copy(dstc, partf[:])
                    else:
                        nc.gpsimd.tensor_tensor(dstc, dstc, partf[:], ALU.add)

            emit_energy(last=False)
            emit_energy(last=False)
            emit_energy(last=True)
            nc.vector.tensor_copy(cfs_row_b[:], cfs_row[:])
            nc.vector.tensor_copy(cfs_col_b[:], cfs_col[:])

            # --- CCAM softmax: attn = softmax(-energy) over K=16 ---
            e_sb = pa.tile([16, 256], F32)
            nc.scalar.activation(e_sb[:], e_ps[:], AF.Copy)
            for half in range(2):
                tps = pasm.tile([128, 16], F32, tag="sm")
                nc.tensor.transpose(
                    tps[:], e_sb[:, half * 128:(half + 1) * 128],
                    idf_s[:16, :16])
                e_c = par.tile([128, 16], F32, tag="ec")
                nc.vector.tensor_copy(e_c[:], tps[:])
                mn = par.tile([128, 1], F32, tag="mn")
                nc.vector.tensor_reduce(mn[:], e_c[:], axis=AX.X, op=ALU.min)
                ex = par.tile([128, 16], F32, tag="ex")
                nc.scalar.activation(ex[:], e_c[:], AF.Exp,
                                     bias=mn[:], scale=-1.0)
                sm = par.tile([128, 1], F32, tag="smv")
                nc.vector.tensor_reduce(sm[:], ex[:], axis=AX.X, op=ALU.add)
                rc = par.tile([128, 1], F32, tag="rc")
                nc.vector.reciprocal(rc[:], sm[:])
                nc.vector.tensor_scalar(at_h[half][:], ex[:], rc[:],
                                        float(scale_ccam), ALU.mult, ALU.mult)

            # --- A^T matrices: A^T = at^T @ W^T (both halves accumulated)
            # normal set lands in A4n rows 32*i..32*i+15 (i: q,k,v0,v1)
            for (dsts, wt, nt) in [(("n", 0), wq_s, 1),
                                   (("n", 1), wk_s, 1),
                                   (("n", 2), wv_s, 2),
                                   ((A_s[0],), wqs_s, 1),
                                   ((A_s[1],), wks_s, 1),
                                   ((A_s[2], A_s[3]), wvs_s, 2)]:
                for mt in range(nt):
                    ps = pasm.tile([16, 128], F32, tag="sm")
                    for half in range(2):
                        nc.tensor.matmul(
                            ps[:], at_h[half][:],
                            wt[:, (half * nt + mt) * 128:
                               (half * nt + mt) * 128 + 128],
                            start=(half == 0), stop=(half == 1))
                    if dsts[0] == "n":
                        i4 = dsts[1] + mt
                        nc.scalar.activation(
                            A4n[32 * i4:32 * i4 + 16, :], ps[:], AF.Copy)
                    else:
                        nc.scalar.activation(dsts[mt][:], ps[:], AF.Copy)

        # =========================================================
        # Region 2: qkv, depthwise+pointwise, axial attn, final
        # =========================================================
        with (
            tc.tile_pool(name="pv", bufs=1) as pv,
            tc.tile_pool(name="pb", bufs=1) as pb,
            tc.tile_pool(name="pbr", bufs=3) as pbr,
        ):
            pqk_cm = tc.tile_pool(name="pqk", bufs=2, space="PSUM")
            pqk = pqk_cm.__enter__()
            v_sb = [pv.tile([128, PSZ], BF, tag=f"v{h}", name=f"v{h}")
                    for h in range(2)]
            for t_ in v_sb:
                # zero only the pad cells: rows 0/129, cols {0,1,130,131}
                nc.vector.memset(_ap(t_, 0, [[129 * PST, 2], [1, PST]]), 0.0)
                nc.vector.memset(
                    _ap(t_, PST, [[PST, 128], [130, 2], [1, 2]]), 0.0)

            # ---- qkv production: 512-col PSUM chunks, 2048-col DMA blocks.
            # The 4 rank-16 corrections are issued back-to-back at distinct
            # 32-row strips -> run concurrently on the PE subarrays.
            # q/k extracted by scalar (-> DRAM); v0/v1 by DVE (-> padded
            # v_sb), so extraction never gates the matmul stream.
            WSPEC = [(wq_s, 1, 0, B_Q), (wk_s, 1, 0, B_K),
                     (wv_s, 2, 0, B_V), (wv_s, 2, 1, B_V + 1)]

            def load_qblock(b):
                sl = slice(b * 2048, (b + 1) * 2048)
                cbi = [pbr.tile([128, 2048], BF, tag=f"cbi{h}",
                                name=f"cbi{h}", bufs=2) for h in range(2)]
                nc.sync.dma_start(cbi[0][:], cb_dram[0, :, sl])
                nc.sync.dma_start(cbi[1][:], cb_dram[1, :, sl])
                cfi = pbr.tile([128, 2048], BF, tag="cfi", bufs=1)
                for i4 in range(4):
                    nc.sync.dma_start(cfi[32 * i4:32 * i4 + 16, :],
                                      cf_dram[:, sl])
                return cbi, cfi

            # zero the DRAM halo rows (image rows -1 and 128) of padded q/k
            zrow = pbr.tile([128, PST], BF, tag="zrow", bufs=1)
            nc.vector.memset(zrow[:], 0.0)
            for t in range(2):
                nc.sync.dma_start(qk_dram[t, :, 0:PST], zrow[:])
                nc.sync.dma_start(qk_dram[t, :, 129 * PST:130 * PST],
                                  zrow[:])
            # pre-zero the qkst ring slot (pad columns persist across reuse)
            qkst_slot = [pbr.tile([128, 16 * PST], BF, tag=f"qkst{t}",
                                  name=f"qkst{t}", bufs=1) for t in range(2)]
            for t_ in qkst_slot:
                nc.vector.memset(t_[:], 0.0)

            qblocks = {0: load_qblock(0)}
            for b in range(8):
                if b + 1 < 8:
                    qblocks[b + 1] = load_qblock(b + 1)
                cbi, cfi = qblocks.pop(b)
                qkst = [pbr.tile([128, 16 * PST], BF, tag=f"qkst{t}",
                                 name=f"qkst{t}", bufs=1) for t in range(2)]
                for sub in range(4):
                    off = sub * 512
                    pr = b * 4 + sub
                    ps = [pqk.tile([128, 512], F32, tag=f"qps{i}",
                                   name=f"qps{i}") for i in range(4)]
                    for i4, (wt, nt, mt, _) in enumerate(WSPEC):
                        for kh in range(2):
                            nc.tensor.matmul(
                                ps[i4][:],
                                wt[:, (kh * nt + mt) * 128:
                                   (kh * nt + mt) * 128 + 128],
                                cbi[kh][:, off:off + 512],
                                start=(kh == 0), stop=False)
                    for i4 in range(4):
                        nc.tensor.matmul(
                            ps[i4][:], A4n[32 * i4:32 * i4 + 16, :],
                            cfi[32 * i4:32 * i4 + 16, off:off + 512],
                            start=False, stop=True,
                            tile_position=(32 * i4, 0))
                    # alternate extraction engines by chunk parity to keep
                    # scalar and DVE equally loaded
                    for t in range(2):
                        qdst = _ap(qkst[t], 4 * sub * PST + 2,
                                   [[PST, 4], [1, 128]])
                        if (pr + t) % 2 == 0:
                            nc.scalar.activation(
                                qdst, ps[t][:], AF.Identity,
                                bias=bia_s[:, WSPEC[t][3]:WSPEC[t][3] + 1])
                        else:
                            nc.vector.tensor_scalar(
                                qdst, ps[t][:],
                                bia_s[:, WSPEC[t][3]:WSPEC[t][3] + 1],
                                None, ALU.add)
                    for mt in range(2):
                        pdst = _ap(v_sb[mt], (4 * pr + 1) * PST + 2,
                                   [[PST, 4], [1, 128]])
                        if (pr + mt) % 2 == 0:
                            nc.vector.tensor_scalar(
                                pdst, ps[2 + mt][:],
                                bia_s[:, B_V + mt:B_V + mt + 1], None,
                                ALU.add)
                        else:
                            nc.scalar.activation(
                                pdst, ps[2 + mt][:], AF.Identity,
                                bias=bia_s[:, B_V + mt:B_V + mt + 1])
                for t in range(2):
                    nc.sync.dma_start(
                        qk_dram[t, :, (b * 16 + 1) * PST:
                                (b * 16 + 17) * PST], qkst[t][:])

            pqk_cm.__exit__(None, None, None)
            pbmm_cm = tc.tile_pool(name="pbmm", bufs=2, space="PSUM")
            pbmm = pbmm_cm.__enter__()
            pcm_cm = tc.tile_pool(name="pcm", bufs=1, space="PSUM")
            pcm = pcm_cm.__enter__()

            # ---- DVE depthwise FMA chains (fills V during qkv/C1) ----
            # per 24-row chunk: 9-tap STT chain into acc, then one
            # bias+relu extraction of the whole chunk (strips pads).
            dve_dw = {}  # t -> list of (c0, crows, chunk-output tile)

            def dve_chain(t, c0):
                vsrc = v_sb[t - 2]
                crows = min(24, 128 - c0)
                start = (c0 + 1) * PST + 2
                nn = crows * PST - 4
                acc = pbr.tile([128, 24 * PST], BF, tag=f"dacc{t}",
                               name=f"dacc{t}", bufs=1)
                acc_ap = _ap(acc, 0, [[1, nn]])
                nc.vector.tensor_scalar(
                    acc_ap, _ap(vsrc, start, [[1, nn]]),
                    dwsc_s[:, t * 9 + 4:t * 9 + 5], None, ALU.mult)
                for (ky, kx) in taps[1:]:
                    tap9 = ky * 3 + kx
                    delta = (ky - 1) * PST + (kx - 1)
                    src = _ap(vsrc, start + delta, [[1, nn]])
                    nc.vector.scalar_tensor_tensor(
                        acc_ap, src,
                        dwsc_s[:, t * 9 + tap9:t * 9 + tap9 + 1],
                        acc_ap, ALU.mult, ALU.add)
                return crows, acc

            def dve_extract(t, c0, crows, acc, outs):
                # extract in 12-row pieces (smaller resident footprint)
                for s12 in range(0, crows, 12):
                    rows = min(12, crows - s12)
                    dwc = pbr.tile([128, 12 * 128], BF, tag=f"dwc{t}",
                                   name=f"dwc{t}", bufs=2)
                    nc.vector.tensor_scalar(
                        dwc[:, 0:rows * 128],
                        _ap(acc, s12 * PST, [[PST, rows], [1, 128]]),
                        bia_s[:, B_DW + t:B_DW + t + 1], 0.0,
                        ALU.add, ALU.max)
                    outs.append((c0 + s12, dwc))

            for t in range(DW_TENSOR_GROUPS, 4):
                dve_dw[t] = []
            dve_done = [0]

            def ensure_dve(upto_row):
                # lazily emit DVE depthwise chains just ahead of consumption
                while dve_done[0] < 128 and dve_done[0] <= upto_row:
                    c0 = dve_done[0]
                    for t in range(DW_TENSOR_GROUPS, 4):
                        crows, acc = dve_chain(t, c0)
                        dve_extract(t, c0, crows, acc, dve_dw[t])
                    dve_done[0] += 24

            # ---- C1 axial attention ----
            xfs_cb = [pb.tile([128, 512], BF, tag=f"xfcb{h}",
                              name=f"xfcb{h}") for h in range(2)]
            for hh in range(2):
                nc.gpsimd.tensor_copy(xfs_cb[hh][:], xfs_col[hh][:])
            for d_ in range(2):
                xfs = xfs_row if d_ == 0 else xfs_cb
                cfs_b = cfs_row_b if d_ == 0 else cfs_col_b
                qs_att = pb.tile([128, 512], BF, tag="qsa", bufs=2)
                ks_att = pb.tile([128, 512], BF, tag="ksa", bufs=2)
                vs_att = [pb.tile([128, 512], BF, tag=f"vsa{h}",
                                  name=f"vsa{h}", bufs=2) for h in range(2)]
                for (dst, wt, As_i, bc, nt, pidx) in [
                        ([qs_att], wqs_s, (0,), B_Q, 1, 2 * d_),
                        ([ks_att], wks_s, (1,), B_K, 1, 2 * d_ + 1),
                        (vs_att, wvs_s, (2, 3), B_V, 2, None)]:
                    for mt in range(nt):
                        ps = pcm.tile([128, 512], F32, tag="cmm")
                        for kh in range(2):
                            nc.tensor.matmul(
                                ps[:],
                                wt[:, (kh * nt + mt) * 128:
                                   (kh * nt + mt) * 128 + 128],
                                xfs[kh][:], start=(kh == 0), stop=False)
                        nc.tensor.matmul(ps[:], A_s[As_i[mt]][:], cfs_b[:],
                                         start=False, stop=(pidx is None))
                        if pidx is not None:
                            for i in range(CH):
                                nc.tensor.matmul(
                                    ps[:, i * 128:(i + 1) * 128],
                                    post_s[:, (pidx * 4 + i) * 128:
                                           (pidx * 4 + i) * 128 + 128],
                                    interp_s[:], start=False, stop=(i == 3))
                        nc.scalar.activation(
                            dst[mt][:], ps[:], AF.Identity,
                            bias=bia_s[:, bc + mt:bc + mt + 1])

                # repack q/k: 4 heads per 32-partition row group
                q_pack = pb.tile([128, 1024], BF, tag="qp", name="qp", bufs=1)
                k_pack = pb.tile([128, 1024], BF, tag="kp", name="kp", bufs=1)
                for g in range(8):
                    po, co = 32 * (g % 4), (g // 4) * 512
                    nc.sync.dma_start(
                        q_pack[po:po + 16, co:co + 512],
                        qs_att[g * 16:(g + 1) * 16, :])
                    nc.sync.dma_start(
                        k_pack[po:po + 16, co:co + 512],
                        ks_att[g * 16:(g + 1) * 16, :])

                # v^T per chunk: [128(pos), i, 256(ch2)]
                vt_s = pb.tile([128, 4, 256], BF, tag="vt", bufs=2)
                for i in range(CH):
                    for hh in range(2):
                        tp = pcm.tile([128, 128], BF, tag="lps")
                        nc.tensor.transpose(
                            tp[:], vs_att[hh][:, i * 128:(i + 1) * 128],
                            idb_s[:])
                        nc.scalar.activation(
                            vt_s[:, i, hh * 128:(hh + 1) * 128], tp[:],
                            AF.Copy)

                xpre = [pb.tile([128, 512], BF, tag=f"xpre{t}",
                                name=f"xpre{t}", bufs=2) for t in range(2)]
                for i in range(CH):
                    for th in range(2):
                        asm_ps = pcm.tile([128, 128], BF, tag="asm")
                        for gg in range(4):
                            g = th * 4 + gg
                            po = 32 * (g % 4)
                            co = (g // 4) * 512
                            sl_gi = slice(co + i * 128, co + i * 128 + 128)
                            l_ps = pcm.tile([128, 128], F32, tag="lps")
                            nc.tensor.matmul(l_ps[:],
                                             k_pack[po:po + 16, sl_gi],
                                             q_pack[po:po + 16, sl_gi],
                                             start=True, stop=True,
                                             tile_position=(po, 0))
                            e_t = pbr.tile([128, 128], BF, tag="et", bufs=2)
                            nc.scalar.activation(e_t[:], l_ps[:], AF.Exp,
                                                 scale=SCALE)
                            av_ps = pcm.tile([128, 33], F32, tag="av")
                            nc.tensor.matmul(
                                av_ps[:, 0:32], e_t[:],
                                vt_s[:, i, g * 32:(g + 1) * 32],
                                start=True, stop=False)
                            nc.tensor.matmul(av_ps[:, 32:33], e_t[:],
                                             ones_s[:], start=False,
                                             stop=True)
                            rcp = pbr.tile([128, 1], F32, tag="rcp")
                            nc.vector.reciprocal(rcp[:], av_ps[:, 32:33])
                            xrn = pbr.tile([128, 32], BF, tag="xrn")
                            with nc.allow_low_precision(
                                    reason="bf16 attn normalize"):
                                nc.vector.tensor_scalar(
                                    xrn[:], av_ps[:, 0:32], rcp[:], None,
                                    ALU.mult)
                            nc.tensor.transpose(
                                asm_ps[gg * 32:(gg + 1) * 32, :], xrn[:],
                                idb_s[:], tile_position=(0, gg * 32))
                        nc.scalar.activation(
                            xpre[th][:, i * 128:(i + 1) * 128], asm_ps[:],
                            AF.Relu)

                wproj_d = wrow_s if d_ == 0 else wcol_s
                bcol = B_ROW if d_ == 0 else B_COL
                for mt in range(2):
                    ps = pcm.tile([128, 512], F32, tag="cmm")
                    for kh in range(2):
                        nc.tensor.matmul(
                            ps[:],
                            wproj_d[:, (kh * 2 + mt) * 128:
                                    (kh * 2 + mt) * 128 + 128],
                            xpre[kh][:], start=(kh == 0), stop=(kh == 1))
                    nc.scalar.activation(
                        xproj[(d_, mt)][:], ps[:], AF.Identity,
                        bias=bia_s[:, bcol + mt:bcol + mt + 1])

            pcm_cm.__exit__(None, None, None)
            pe2_cm = tc.tile_pool(name="pe2", bufs=2, space="PSUM")
            pe2 = pe2_cm.__enter__()

            # ---- depthwise 3x3 ----
            dwd_s = pb.tile([128, 36 * 128], BF)
            nc.sync.dma_start(dwd_s[:], dwd[:])
            dblk = [(r0, 3) for r0 in range(0, 126, 3)] + [(126, 2)]

            def dve_chunk_of(r0):
                return r0 // 24

            qo_tiles = {}

            def pair_of_row(r):
                return min(r // 6, 21)

            def emit_gate(bb):
                # xx = relu(v + bcast(xrow) + bcast(xcol));
                # att = hsig(proj(xx) + b + 3); out = att * qkv2
                xxh = []
                for half in range(2):
                    xx = pbr.tile([128, BL], BF, tag=f"xx{half}",
                                  name=f"xx{half}", bufs=3)
                    rap = _ap(xproj[(0, half)], bb * 16, [[1, 16], [0, 32]])
                    cap = _ap(xproj[(1, half)], (bb // 2) * 32,
                              [[0, 4], [0, 4], [1, 32]])
                    nc.vector.tensor_tensor(xx[:], rap, cap, ALU.add)
                    vap = _ap(v_sb[half], (4 * bb + 1) * PST + 2,
                              [[PST, 4], [1, 128]])
                    nc.vector.tensor_tensor(xx[:], xx[:], vap, ALU.add)
                    nc.vector.tensor_scalar(xx[:], xx[:], 0.0, None,
                                            ALU.max)
                    xxh.append(xx)
                r0b = 4 * bb
                p0, p1 = pair_of_row(r0b), pair_of_row(r0b + 3)
                segs = ([(p0, 0, 4)] if p0 == p1 else
                        [(p0, 0, 6 * p1 - r0b), (p1, 6 * p1 - r0b, 4)])
                for mt in range(2):
                    ps = pe2.tile([128, 512], F32, tag="jps", name="jps")
                    for kh in range(2):
                        nc.tensor.matmul(
                            ps[:],
                            wproj_s[:, (kh * 2 + mt) * 128:
                                    (kh * 2 + mt) * 128 + 128],
                            xxh[kh][:], start=(kh == 0), stop=(kh == 1))
                    hs = pbr.tile([128, BL], BF, tag="hs", bufs=2)
                    nc.scalar.activation(
                        hs[:], ps[:], AF.Relu,
                        bias=bia_s[:, B_PROJ3 + mt:B_PROJ3 + mt + 1])
                    att_t = pbr.tile([128, BL], BF, tag="att", bufs=2)
                    nc.vector.tensor_scalar(att_t[:], hs[:], 6.0, 1.0 / 6.0,
                                            ALU.min, ALU.mult)
                    for (p, lo, hi) in segs:
                        nr_s = hi - lo
                        qo = qo_tiles[(p, mt)]
                        ob = pbr.tile([128, BL], BF, tag="ob", bufs=2)
                        nc.vector.tensor_tensor(
                            ob[:, 0:nr_s * 128],
                            att_t[:, lo * 128:hi * 128],
                            qo[:, (r0b + lo - 6 * p) * 128:
                               (r0b + hi - 6 * p) * 128], ALU.mult)
                        nc.sync.dma_start(
                            out[mt * 128:(mt + 1) * 128,
                                (r0b + lo) * 128:(r0b + hi) * 128],
                            ob[:, 0:nr_s * 128])

            # superblocks of up to 4 dblk groups (12 rows); diag weight
            # loads amortize over all 4 groups' matmuls per tap.
            next_gate = [0]
            for g0 in range(0, len(dblk), 4):
                grp4 = dblk[g0:g0 + 4]
                pairs = [p for p in (grp4[0:2], grp4[2:4]) if p]
                ensure_dve(grp4[0][0] + 23)
                # q/k windows per pair (padded rows, contiguous in DRAM)
                wins_p = []
                for t in range(min(DW_TENSOR_GROUPS, 2)):
                    wp = []
                    for grp in pairs:
                        rp = grp[0][0]
                        nrp = sum(nr for _, nr in grp)
                        win = pbr.tile([128, 8 * PST], BF, tag=f"win{t}",
                                       name=f"win{t}", bufs=2)
                        nc.sync.dma_start(
                            _ap(win, 0, [[1, (nrp + 2) * PST]]),
                            qk_dram[t, :, rp * PST:(rp + nrp + 2) * PST])
                        wp.append(win)
                    wins_p.append(wp)

                dwg = []  # dwg[t] = per-group output slices (global j)
                for t in range(DW_TENSOR_GROUPS):
                    pstiles = [pbmm.tile([128, 1024], F32, tag="bmm",
                                         name="bmm") for _ in pairs]
                    for tt, (ky, kx) in enumerate(taps):
                        tap9 = ky * 3 + kx
                        wsl = dwd_s[:, (t * 9 + tap9) * 128:
                                    (t * 9 + tap9) * 128 + 128]
                        for pi_, grp in enumerate(pairs):
                            rp = grp[0][0]
                            for j, (r0, nr) in enumerate(grp):
                                nn = nr * PST - 4
                                if t < 2:
                                    rhs = _ap(wins_p[t][pi_],
                                              (r0 - rp + ky) * PST + kx + 1,
                                              [[1, nn]])
                                else:
                                    rhs = _ap(v_sb[t - 2],
                                              (r0 + ky) * PST + kx + 1,
                                              [[1, nn]])
                                nc.tensor.matmul(
                                    _ap(pstiles[pi_], j * 512, [[1, nn]]),
                                    wsl, rhs, start=(tt == 0),
                                    stop=(tt == 8))
                    dwt_all = []
                    for pi_, grp in enumerate(pairs):
                        for j, (r0, nr) in enumerate(grp):
                            gj = 2 * pi_ + j
                            dwt = pbr.tile([128, 384], BF,
                                           tag=f"dw{t}{gj}",
                                           name=f"dw{t}{gj}", bufs=1)
                            nc.scalar.activation(
                                dwt[:, 0:nr * 128],
                                _ap(pstiles[pi_], j * 512,
                                    [[PST, nr], [1, 128]]),
                                AF.Relu,
                                bias=bia_s[:, B_DW + t:B_DW + t + 1])
                            dwt_all.append(dwt)
                    dwg.append(dwt_all)
                for t in range(DW_TENSOR_GROUPS, 4):
                    slc = []
                    for pi_, grp in enumerate(pairs):
                        for j, (r0, nr) in enumerate(grp):
                            b0, dwc = dve_dw[t][r0 // 12]
                            slc.append(dwc[:, (r0 - b0) * 128:
                                           (r0 - b0 + nr) * 128])
                    dwg.append(slc)

                # pointwise + gating per pair
                for pi_, grp in enumerate(pairs):
                    rp = grp[0][0]
                    nrp = sum(nr for _, nr in grp)
                    for mt in range(2):
                        ps = pbmm.tile([128, 1024], F32, tag="bmm",
                                       name="pwm")
                        for kt in range(4):
                            wsl = wpw_s[:, kt * 256 + mt * 128:
                                        kt * 256 + mt * 128 + 128]
                            for j, (r0, nr) in enumerate(grp):
                                gj = 2 * pi_ + j
                                rhs = (dwg[kt][gj][:, 0:nr * 128]
                                       if kt < DW_TENSOR_GROUPS
                                       else dwg[kt][gj])
                                nc.tensor.matmul(
                                    ps[:, j * 512:j * 512 + nr * 128],
                                    wsl, rhs, start=(kt == 0),
                                    stop=(kt == 3))
                        qo = pbr.tile([128, 768], BF, tag=f"qo{mt}",
                                      name=f"qo{mt}", bufs=2)
                        if len(grp) == 2:
                            src = _ap(ps, 0,
                                      [[512, 2], [1, grp[0][1] * 128]])
                        else:
                            src = _ap(ps, 0, [[1, grp[0][1] * 128]])
                        nc.scalar.activation(
                            qo[:, 0:nrp * 128], src, AF.Identity,
                            bias=bia_s[:, B_PW + mt:B_PW + mt + 1])
                        qo_tiles[(g0 // 2 + pi_, mt)] = qo

                    pi = g0 // 2 + pi_
                    while next_gate[0] < 32 and \
                            pair_of_row(4 * next_gate[0] + 3) <= pi:
                        emit_gate(next_gate[0])
                        next_gate[0] += 1

            pe2_cm.__exit__(None, None, None)
            pbmm_cm.__exit__(None, None, None)

    nc.compile()
    return nc


def _interp_matrix():
    s, n = 16, 128
    src = np.clip((np.arange(n) + 0.5) * (s / n) - 0.5, 0.0, s - 1.0)
    i0 = np.floor(src).astype(np.int64)
    i1 = np.minimum(i0 + 1, s - 1)
    w = src - i0
    M = np.zeros((s, n), np.float64)
    np.add.at(M, (i0, np.arange(n)), 1.0 - w)
    np.add.at(M, (i1, np.arange(n)), w)
    return M


def _bf(x):
    return np.ascontiguousarray(np.asarray(x, np.float32).astype(
        ml_dtypes.bfloat16))


def prep_consts(inputs):
    """Host-side layout prep of all weight tensors (shared across cores)."""
    f = {k: np.asarray(v, np.float32) for k, v in inputs.items()}

    w3 = f["w_ccam_b"]                      # [256, 128, 3, 3]
    w3t = np.zeros((128, 9 * 256), np.float32)
    for ky in range(3):
        for kx in range(3):
            t9 = ky * 3 + kx
            w3t[:, t9 * 256:(t9 + 1) * 256] = w3[:, :, ky, kx].T
    wenc = np.zeros((128, 32), np.float32)  # w_enc [16, 256]
    for half in range(2):
        wenc[:, half * 16:(half + 1) * 16] = \
            f["w_enc"][:, half * 128:(half + 1) * 128].T

    def pack_lhsT(wm, nt):
        # wm [out, in]; returns [128, 2*nt*128]: [ci, (kh*nt+mt)*128+co]
        o, cin = wm.shape
        r = np.zeros((128, 2 * nt * 128), np.float32)
        for kh in range(2):
            for mt in range(nt):
                r[:, (kh * nt + mt) * 128:(kh * nt + mt) * 128 + 128] = \
                    wm[mt * 128:(mt + 1) * 128,
                       kh * 128:(kh + 1) * 128].T
        return r

    wq_p = pack_lhsT(f["w_q"], 1)
    wk_p = pack_lhsT(f["w_k"], 1)
    wv_p = pack_lhsT(f["w_v"], 2)
    wrow_p = pack_lhsT(f["w_row"], 2)
    wcol_p = pack_lhsT(f["w_col"], 2)
    wproj_p = pack_lhsT(f["w_proj"], 2)

    wpw_p = np.zeros((128, 4 * 256), np.float32)   # w_pw [256, 512]
    for kt in range(4):
        for mt in range(2):
            wpw_p[:, kt * 256 + mt * 128:kt * 256 + mt * 128 + 128] = \
                f["w_pw"][mt * 128:(mt + 1) * 128,
                          kt * 128:(kt + 1) * 128].T

    dwdg = np.zeros((128, 36 * 128), np.float32)   # w_dw [512,1,3,3]
    ii = np.arange(128)
    for t in range(4):
        for tap9 in range(9):
            ky, kx = divmod(tap9, 3)
            dwdg[ii, (t * 9 + tap9) * 128 + ii] = \
                f["w_dw"][t * 128 + ii, 0, ky, kx]

    post_p = np.zeros((16, 4 * 512), np.float32)
    for pidx, nm in enumerate(["pos_rowq", "pos_rowk", "pos_colq", "pos_colk"]):
        p = f[nm]                                   # [4, 128, 16]
        for i in range(4):
            post_p[:, (pidx * 4 + i) * 128:(pidx * 4 + i) * 128 + 128] = \
                p[i].T                              # [16, 128]

    biases = np.zeros((128, 20), np.float32)
    biases[:, B_CCAM + 0] = f["b_ccam_b"][:128]
    biases[:, B_CCAM + 1] = f["b_ccam_b"][128:]
    biases[:16, B_ENC] = f["b_enc"]
    biases[:, B_Q] = f["b_q"]
    biases[:, B_K] = f["b_k"]
    biases[:, B_V + 0] = f["b_v"][:128]
    biases[:, B_V + 1] = f["b_v"][128:]
    for t in range(4):
        biases[:, B_DW + t] = f["b_dw"][t * 128:(t + 1) * 128]
    biases[:, B_PW + 0] = f["b_pw"][:128]
    biases[:, B_PW + 1] = f["b_pw"][128:]
    biases[:, B_ROW + 0] = f["b_row"][:128]
    biases[:, B_ROW + 1] = f["b_row"][128:]
    biases[:, B_COL + 0] = f["b_col"][:128]
    biases[:, B_COL + 1] = f["b_col"][128:]
    biases[:, B_PROJ3 + 0] = f["b_proj"][:128] + 3.0
    biases[:, B_PROJ3 + 1] = f["b_proj"][128:] + 3.0

    dwsc_p = np.zeros((128, 36), np.float32)
    for t in range(4):
        for tap9 in range(9):
            ky, kx = divmod(tap9, 3)
            dwsc_p[:, t * 9 + tap9] = f["w_dw"][t * 128:(t + 1) * 128,
                                                0, ky, kx]
    return {
        "dwsc": np.ascontiguousarray(dwsc_p),
        "w3t": _bf(w3t), "wenc": _bf(wenc),
        "wq": _bf(wq_p), "wk": _bf(wk_p), "wv": _bf(wv_p),
        "wqs": _bf(wq_p / 32.0), "wks": _bf(wk_p / 32.0),
        "wvs": _bf(wv_p / 32.0),
        "dwd": _bf(dwdg), "wpw": _bf(wpw_p),
        "wrow": _bf(wrow_p), "wcol": _bf(wcol_p), "wproj": _bf(wproj_p),
        "post": _bf(post_p), "interpm": _bf(_interp_matrix()),
        "identb": _bf(np.eye(128)),
        "identf": np.eye(128, dtype=np.float32),
        "onesb": _bf(np.ones((128, 1))),
        "biases": np.ascontiguousarray(biases),
    }


def kernel(**inputs) -> np.ndarray:
    x = np.asarray(inputs["x"], np.float32)          # [8, 128, 128, 128]
    scale = float(np.asarray(inputs["scale_ccam"]).reshape(-1)[0])

    key = round(scale, 9)
    if key not in _CACHE:
        _CACHE[key] = build_graph(scale)
    nc = _CACHE[key]

    consts = prep_consts(inputs)
    in_maps = []
    for core in range(8):
        m = dict(consts)
        m["xb"] = _bf(x[core].reshape(128, N))
        in_maps.append(m)

    res = run_bass_kernel_spmd(nc, in_maps, core_ids=list(range(8)))
    outs = [np.asarray(res.results[i]["out"]).astype(np.float32)
            .reshape(256, 128, 128) for i in range(8)]
    return np.stack(outs)


if __name__ == "__main__":
    rng = np.random.default_rng(0)
    demo = {"x": rng.standard_normal((8, 128, 128, 128), dtype=np.float32)}
    print("kernel module OK")

